# revision 1
# baseline (speedup 1.0000x reference)
"""BackpropWiSARD forward on 8 Trainium2 NeuronCores.

Strategy (filter-sharded):
  out[b,c] = sum_f mask[c,f] * min_h [data[c, f, idx[b,f,h]] >= 0] + bias[c]

- Host computes the hashed indices idx[b,f,h] (tiny: binarize x, permute,
  XOR-hash), and re-lays-out the 420MB table as [F, E, Cpad=128] bf16 rows so
  one gathered 256B row holds all classes for one (f, e).
- The filter axis F=512 is sharded 64-per-core. Each core gathers, for its
  filters, the B*H rows it needs straight from HBM via GPSIMD dma_gather
  (16 filters = 16384 row-gathers per instruction; row index = f_local*2048+e
  fits int16 exactly), then on-chip: min over the H=4 hash lookups (min and
  binarize commute since x>=0 is monotone), binarize+mask via one
  scalar_tensor_tensor, reduce over filters, accumulate.
- Each core returns a partial [b, c] sum over its filters; the host adds the
  8 partials and the bias (pure unshard/combine work).
"""

import numpy as np
import ml_dtypes

B = 256      # batch
NI = 1024    # num inputs
C = 100      # classes
U = 16       # unit inputs
E = 2048     # unit entries
H = 4        # hashes
BPI = 8      # bits per input
IB = NI * BPI          # 8192
F = IB // U            # 512 filters
NCORES = 8
FPC = F // NCORES      # 64 filters per core
CP = 128               # padded class dim (256B bf16 rows)
GF = 8                 # filters per gather group (HW dma_gather limit: 8192 idxs)
NG = FPC // GF         # 8 groups
NIDX = GF * H * B      # 8192 gathered rows per group
BQ = B // 128          # 2 partition-blocks of the batch
NCHUNK = 8             # gather chunks per group (4 SWDGE queues, depth 8)

_NC = {}


def _build_nc(reps=1, variant='full'):
    import os
    STAGES = os.environ.get("WISARD_STAGES", "12345")
    from contextlib import ExitStack
    import concourse.bacc as bacc
    import concourse.mybir as mybir

    nc = bacc.Bacc("TRN2", target_bir_lowering=False, debug=False,
                   num_devices=NCORES, dynamic_dma_scratch_size=32768,
                   num_swdge_queues=4)
    table = nc.dram_tensor("table", [FPC * E, CP], mybir.dt.bfloat16,
                           kind="ExternalInput")
    idxw = nc.dram_tensor("idxw", [128, NG * (NIDX // 16)], mybir.dt.int16,
                          kind="ExternalInput")
    maskr = nc.dram_tensor("maskr", [128, FPC * CP], mybir.dt.float32,
                           kind="ExternalInput")
    out_acc = nc.dram_tensor("out_acc", [128, BQ * CP], mybir.dt.float32,
                             kind="ExternalOutput")

    mn = mybir.AluOpType.min
    with ExitStack() as sem_stack:
        ent = sem_stack.enter_context
        idx_sb = ent(nc.sbuf_tensor("idx_sb", [128, NG * (NIDX // 16)], mybir.dt.int16))
        mask_sb = ent(nc.sbuf_tensor("mask_sb", [128, FPC * CP], mybir.dt.float32))
        gt0 = ent(nc.sbuf_tensor("gt0", [128, NIDX], mybir.dt.bfloat16))
        gt1 = ent(nc.sbuf_tensor("gt1", [128, NIDX], mybir.dt.bfloat16))
        gt2 = ent(nc.sbuf_tensor("gt2", [128, NIDX], mybir.dt.bfloat16))
        gt3 = ent(nc.sbuf_tensor("gt3", [128, NIDX], mybir.dt.bfloat16))
        mask_bf = ent(nc.sbuf_tensor("mask_bf", [128, FPC * CP], mybir.dt.bfloat16))
        mA0 = ent(nc.sbuf_tensor("mA0", [128, 2 * BQ * GF * CP], mybir.dt.bfloat16))
        mA1 = ent(nc.sbuf_tensor("mA1", [128, 2 * BQ * GF * CP], mybir.dt.bfloat16))
        mA2 = ent(nc.sbuf_tensor("mA2", [128, 2 * BQ * GF * CP], mybir.dt.bfloat16))
        mA3 = ent(nc.sbuf_tensor("mA3", [128, 2 * BQ * GF * CP], mybir.dt.bfloat16))
        mB0 = ent(nc.sbuf_tensor("mB0", [128, BQ * GF * CP], mybir.dt.bfloat16))
        mB1 = ent(nc.sbuf_tensor("mB1", [128, BQ * GF * CP], mybir.dt.bfloat16))
        rm0 = ent(nc.sbuf_tensor("rm0", [128, BQ * GF * CP], mybir.dt.bfloat16))
        rm1 = ent(nc.sbuf_tensor("rm1", [128, BQ * GF * CP], mybir.dt.bfloat16))
        red0 = ent(nc.sbuf_tensor("red0", [128, BQ * CP], mybir.dt.float32))
        red1 = ent(nc.sbuf_tensor("red1", [128, BQ * CP], mybir.dt.float32))
        acc = ent(nc.sbuf_tensor("acc", [128, BQ * CP], mybir.dt.float32))
        s_in = ent(nc.semaphore("s_in"))
        s_v = ent(nc.semaphore("s_v"))
        s_f = ent(nc.semaphore("s_f"))
        s_g = [[sem_stack.enter_context(nc.semaphore(f"s_g{g}q{q}"))
                for q in range(4)] for g in range(NG)]
        gts = [gt0, gt1, gt2, gt3]
        NBUF = len(gts)
        mAs, mBs, rms, reds = [mA0, mA1, mA2, mA3], [mB0, mB1], [rm0, rm1], [red0, red1]
        GCOLS = GF * CP            # 1024 cols per (h, B') block... per B' block
        HB = 2 * GCOLS             # 2048 cols per h block
        NIT = NG + 4               # pipeline iterations per rep

        # --- gpsimd: input loads + the gather groups ---------------------
        # (Bacc auto-inserts the GPSIMD 'mlp' library load for dma_gather)
        nc.gpsimd.dma_start(idx_sb[:, :], idxw[:, :]).then_inc(s_in, 16)
        nc.gpsimd.dma_start(mask_sb[:, :], maskr[:, :]).then_inc(s_in, 16)
        nc.gpsimd.wait_ge(s_in, 32)
        gather_reps = reps if variant in ('full', 'gather_only') else 1
        dve_reps = reps if variant in ('full', 'dve_only') else 1
        CH = NIDX // NCHUNK  # idxs per gather chunk
        for rep in range(gather_reps):
            for g in range(NG):
                j = rep * NG + g
                buf = gts[j % NBUF]
                if variant == 'full' and j >= NBUF:
                    # wait for the pipeline iteration whose stage-1 (the h-min
                    # that is the last gt reader) consumed group j-NBUF
                    jp = j - NBUF
                    it_of = (jp // NG) * NIT + (jp % NG)
                    nc.gpsimd.wait_ge(s_v, it_of + 1)
                for ch in range(NCHUNK):
                    # chunk ch covers group idxs [ch*CH, (ch+1)*CH): same
                    # wrapped-idx cols and same gt cols as one big gather
                    nc.gpsimd.dma_gather(
                        buf[:, ch * CH:(ch + 1) * CH].rearrange(
                            "p (j c) -> p j c", c=CP),
                        table[g * GF * E:(g + 1) * GF * E, :],
                        idx_sb[:, g * (NIDX // 16) + ch * (CH // 16):
                               g * (NIDX // 16) + (ch + 1) * (CH // 16)],
                        CH, CH, CP, single_packet=False,
                        queue_num=ch % 4,
                    ).then_inc(s_g[g][ch % 4], 16)

        # --- vector: software-pipelined chain, one drain per iteration.
        # Gather layout per group: cols = (4h, 2B', 8f, 128c); every op below
        # is a flat contiguous 2D slice (bf16 2x-mode friendly).
        per_q = NCHUNK // 4
        # one-time: mask f32 -> bf16 for all-bf16 deep stages
        nc.vector.wait_ge(s_in, 32)
        nc.vector.tensor_copy(mask_bf[:, :], mask_sb[:, :])
        def dve_iter(rep, k):
            # stage 1 first: h-pair mins from the gathered tile (group k);
            # the drain right after releases the gather buffer ASAP
            if '1' in STAGES and k < NG:
                j = rep * NG + k
                buf = gts[j % NBUF] if variant == 'full' else gts[0]
                for q in range(4):
                    nc.vector.wait_ge(s_g[k][q], 16 * per_q * (rep + 1)
                                      if variant == 'full' else 16 * per_q)
                for m in range(2):
                    nc.vector.tensor_tensor(
                        mAs[k % 4][:, m * HB:(m + 1) * HB],
                        buf[:, m * HB:(m + 1) * HB],
                        buf[:, (m + 2) * HB:(m + 3) * HB], mn)
            nc.vector.drain().then_inc(s_v, 1)
            # stage 5: acc += red  (group k-4)
            if '5' in STAGES and 0 <= k - 4 < NG:
                nc.vector.tensor_tensor(
                    acc[:, :], acc[:, :], reds[(k - 4) % 2][:, :],
                    mybir.AluOpType.add)
            # stage 4: reduce over f  (group k-3)
            if '4' in STAGES and 0 <= k - 3 < NG:
                rm = rms[(k - 3) % 2][:, :].rearrange(
                    "p (q f c) -> p q f c", q=BQ, f=GF)
                nc.vector.tensor_reduce(
                    reds[(k - 3) % 2][:, :].rearrange("p (q c) -> p q c", q=BQ),
                    rm.transpose([0, 1, 3, 2]),
                    mybir.AxisListType.X, mybir.AluOpType.add)
            # stage 3: respm = (min >= 0) * mask  (group k-2)
            if '3' in STAGES and 0 <= k - 2 < NG:
                g2 = k - 2
                mg = mask_bf[:, g2 * GCOLS:(g2 + 1) * GCOLS]
                for q in range(BQ):
                    nc.vector.scalar_tensor_tensor(
                        rms[g2 % 2][:, q * GCOLS:(q + 1) * GCOLS],
                        mBs[g2 % 2][:, q * GCOLS:(q + 1) * GCOLS],
                        0.0, mg,
                        mybir.AluOpType.is_ge, mybir.AluOpType.mult)
            # stage 2: min of the two h-pair minima  (group k-1)
            if '2' in STAGES and 0 <= k - 1 < NG:
                g1 = k - 1
                nc.vector.tensor_tensor(
                    mBs[g1 % 2][:, :], mAs[g1 % 4][:, :HB],
                    mAs[g1 % 4][:, HB:], mn)

        for rep in range(dve_reps):
            nc.vector.memset(acc[:, :], 0.0)
            for k in range(NIT):
                dve_iter(rep, k)
        nc.vector.drain().then_inc(s_f, 1)

        # --- sync: write the partial result back -------------------------
        nc.sync.wait_ge(s_f, 1)
        nc.sync.dma_start(out_acc[:, :], acc[:, :]).then_inc(s_f, 16)
        nc.sync.wait_ge(s_f, 17)
    nc.finalize()
    return nc


def _get_nc(reps=1, variant='full'):
    key = (reps, variant)
    if key not in _NC:
        _NC[key] = _build_nc(reps, variant)
    return _NC[key]


def _hashed_indices(x, thresholds, hash_values, input_order):
    """idx[b, f, h] in [0, E) — the H3 hash of the binarized inputs."""
    bits = (x[:, :, None] >= thresholds[None, :, :])
    bits = bits.reshape(B, IB)[:, input_order].astype(np.int32)
    hin = bits.reshape(B, F, U)
    prod = hin[:, :, None, :] * hash_values[None, None, :, :].astype(np.int32)
    return np.bitwise_xor.reduce(prod, axis=-1)  # [B, F, H]


def _shard_inputs(idx, data, mask):
    """Per-core input dicts: table slab, wrapped gather indices, repl. mask."""
    data_t = np.zeros((F, E, CP), dtype=ml_dtypes.bfloat16)
    data_t[:, :, :C] = np.transpose(data, (1, 2, 0)).astype(ml_dtypes.bfloat16)
    in_maps = []
    for k in range(NCORES):
        fs = k * FPC
        table_k = np.ascontiguousarray(data_t[fs:fs + FPC]).reshape(FPC * E, CP)
        # gather order within a group: i = ((h*BQ + B')*GF + f_local)*128 + p
        # (h outermost so every DVE op is a flat contiguous slice)
        idxk = idx[:, fs:fs + FPC, :]                        # [B, FPC, H]
        r = np.empty((NG, NIDX), np.int32)
        offs = np.arange(GF, dtype=np.int32) * E
        for g in range(NG):
            sub = idxk[:, g * GF:(g + 1) * GF, :]            # [B, GF, H]
            a = sub.reshape(BQ, 128, GF, H).transpose(3, 0, 2, 1)  # [H,BQ,GF,128]
            r[g] = (a + offs[None, None, :, None]).reshape(NIDX)
        iw16 = np.zeros((16, NG * (NIDX // 16)), np.int16)
        for g in range(NG):
            iw16[:, g * (NIDX // 16):(g + 1) * (NIDX // 16)] = (
                r[g].reshape(NIDX // 16, 16).T.astype(np.int16))
        iw = np.tile(iw16, (8, 1))  # replicated per Q7 core group
        mk = np.zeros((FPC, CP), np.float32)
        mk[:, :C] = mask[:, fs:fs + FPC].T
        mrep = np.ascontiguousarray(
            np.broadcast_to(mk.reshape(1, FPC * CP), (128, FPC * CP)))
        in_maps.append({"table": table_k, "idxw": iw, "maskr": mrep})
    return in_maps


def kernel(x, thresholds, data, hash_values, input_order, mask, bias):
    import os
    from concourse.bass_utils import run_bass_kernel_spmd

    x = np.asarray(x, np.float32)
    thresholds = np.asarray(thresholds, np.float32)
    data = np.asarray(data, np.float32)
    hash_values = np.asarray(hash_values, np.int32)
    input_order = np.asarray(input_order, np.int32)
    mask = np.asarray(mask, np.float32)
    bias = np.asarray(bias, np.float32)

    idx = _hashed_indices(x, thresholds, hash_values, input_order)
    in_maps = _shard_inputs(idx, data, mask)

    trace = bool(int(os.environ.get("WISARD_TRACE", "0")))
    res = run_bass_kernel_spmd(_get_nc(), in_maps, core_ids=list(range(NCORES)),
                               trace=trace)
    if trace and res.exec_time_ns is not None:
        kernel.last_exec_time_ns = res.exec_time_ns
        kernel.last_trace = res.instructions_and_trace
    kernel.last_results = res

    out = np.zeros((B, CP), np.float32)
    for r in res.results:
        out += r["out_acc"].reshape(128, BQ, CP).transpose(1, 0, 2).reshape(B, CP)
    return out[:, :C] + bias[None, :].astype(np.float32)



# revision 2
# speedup vs baseline: 1.2580x; 1.2580x over previous
"""BackpropWiSARD forward on 8 Trainium2 NeuronCores.

Strategy (filter-sharded):
  out[b,c] = sum_f mask[c,f] * min_h [data[c, f, idx[b,f,h]] >= 0] + bias[c]

- Host computes the hashed indices idx[b,f,h] (tiny: binarize x, permute,
  XOR-hash), and re-lays-out the 420MB table as [F, E, Cpad=128] bf16 rows so
  one gathered 256B row holds all classes for one (f, e).
- The filter axis F=512 is sharded 64-per-core. Each core gathers, for its
  filters, the B*H rows it needs straight from HBM via GPSIMD dma_gather
  (single_packet=True so each DMA engine's descriptor stream coalesces into
  one packet), then on-chip: min over the H=4 hash lookups, binarize+mask via
  one scalar_tensor_tensor, add-tree over filters, accumulate.
- Inputs (indices, mask) load via the SP engine's HWDGE so the gather queues
  and GPSIMD start immediately; the mask is pre-tiled bf16 on host.
- Each core returns a partial [b, c] sum over its filters; the host adds the
  8 partials and the bias (pure unshard/combine work).
"""

import numpy as np
import ml_dtypes

B = 256      # batch
NI = 1024    # num inputs
C = 100      # classes
U = 16       # unit inputs
E = 2048     # unit entries
H = 4        # hashes
BPI = 8      # bits per input
IB = NI * BPI          # 8192
F = IB // U            # 512 filters
NCORES = 8
FPC = F // NCORES      # 64 filters per core
CP = 128               # padded class dim (256B bf16 rows)
GF = 8                 # filters per gather group (HW dma_gather limit: 8192 idxs)
NG = FPC // GF         # 8 groups
NIDX = GF * H * B      # 8192 gathered rows per group
BQ = B // 128          # 2 partition-blocks of the batch
NCHUNK = 8             # gather chunks per group (4 SWDGE queues)

_NC = {}


def _build_nc(reps=1, variant='full'):
    import os
    STAGES = os.environ.get("WISARD_STAGES", "12345")
    from contextlib import ExitStack
    import concourse.bacc as bacc
    import concourse.mybir as mybir

    nc = bacc.Bacc("TRN2", target_bir_lowering=False, debug=False,
                   num_devices=NCORES, dynamic_dma_scratch_size=32768,
                   num_swdge_queues=4)
    table = nc.dram_tensor("table", [FPC * E, CP], mybir.dt.bfloat16,
                           kind="ExternalInput")
    idxw = nc.dram_tensor("idxw", [128, NG * (NIDX // 16)], mybir.dt.int16,
                          kind="ExternalInput")
    maskr = nc.dram_tensor("maskr", [128, NG * BQ * GF * CP], mybir.dt.bfloat16,
                           kind="ExternalInput")
    out_acc = nc.dram_tensor("out_acc", [128, BQ * CP], mybir.dt.float32,
                             kind="ExternalOutput")

    mn = mybir.AluOpType.min
    ad = mybir.AluOpType.add
    with ExitStack() as sem_stack:
        ent = sem_stack.enter_context
        idx_sb = ent(nc.sbuf_tensor("idx_sb", [128, NG * (NIDX // 16)], mybir.dt.int16))
        mask_sb = ent(nc.sbuf_tensor("mask_sb", [128, NG * BQ * GF * CP], mybir.dt.bfloat16))
        gt0 = ent(nc.sbuf_tensor("gt0", [128, NIDX], mybir.dt.bfloat16))
        gt1 = ent(nc.sbuf_tensor("gt1", [128, NIDX], mybir.dt.bfloat16))
        gt2 = ent(nc.sbuf_tensor("gt2", [128, NIDX], mybir.dt.bfloat16))
        gt3 = ent(nc.sbuf_tensor("gt3", [128, NIDX], mybir.dt.bfloat16))
        mA0 = ent(nc.sbuf_tensor("mA0", [128, 2 * BQ * GF * CP], mybir.dt.bfloat16))
        mA1 = ent(nc.sbuf_tensor("mA1", [128, 2 * BQ * GF * CP], mybir.dt.bfloat16))
        mA2 = ent(nc.sbuf_tensor("mA2", [128, 2 * BQ * GF * CP], mybir.dt.bfloat16))
        mA3 = ent(nc.sbuf_tensor("mA3", [128, 2 * BQ * GF * CP], mybir.dt.bfloat16))
        mB0 = ent(nc.sbuf_tensor("mB0", [128, BQ * GF * CP], mybir.dt.bfloat16))
        mB1 = ent(nc.sbuf_tensor("mB1", [128, BQ * GF * CP], mybir.dt.bfloat16))
        rm0 = ent(nc.sbuf_tensor("rm0", [128, BQ * GF * CP], mybir.dt.bfloat16))
        rm1 = ent(nc.sbuf_tensor("rm1", [128, BQ * GF * CP], mybir.dt.bfloat16))
        u10 = ent(nc.sbuf_tensor("u10", [128, BQ * (GF // 2) * CP], mybir.dt.bfloat16))
        u11 = ent(nc.sbuf_tensor("u11", [128, BQ * (GF // 2) * CP], mybir.dt.bfloat16))
        u20 = ent(nc.sbuf_tensor("u20", [128, BQ * (GF // 4) * CP], mybir.dt.bfloat16))
        u21 = ent(nc.sbuf_tensor("u21", [128, BQ * (GF // 4) * CP], mybir.dt.bfloat16))
        red0 = ent(nc.sbuf_tensor("red0", [128, BQ * CP], mybir.dt.float32))
        red1 = ent(nc.sbuf_tensor("red1", [128, BQ * CP], mybir.dt.float32))
        acc = ent(nc.sbuf_tensor("acc", [128, BQ * CP], mybir.dt.float32))
        s_idx = ent(nc.semaphore("s_idx"))
        s_msk = ent(nc.semaphore("s_msk"))
        s_v = ent(nc.semaphore("s_v"))
        s_f = ent(nc.semaphore("s_f"))
        s_g = [sem_stack.enter_context(nc.semaphore(f"s_g{g}")) for g in range(NG)]
        gts = [gt0, gt1, gt2, gt3]
        NBUF = len(gts)
        mAs, mBs, rms = [mA0, mA1, mA2, mA3], [mB0, mB1], [rm0, rm1]
        u1s, u2s, reds = [u10, u11], [u20, u21], [red0, red1]
        GCOLS = GF * CP            # 1024 cols per (f-block) of one B'
        QCOLS = BQ * GCOLS         # 2048 cols per h block
        HB = QCOLS
        NIT = NG + 4               # pipeline iterations per rep

        # --- input loads via SP-engine HWDGE (fast, off the gather queues) --
        nc.sync.dma_start(idx_sb[:, :], idxw[:, :]).then_inc(s_idx, 16)
        nc.sync.dma_start(mask_sb[:, :], maskr[:, :]).then_inc(s_msk, 16)

        # --- gpsimd: the gather groups ----------------------------------
        # (Bacc auto-inserts the GPSIMD 'mlp' library load for dma_gather)
        nc.gpsimd.wait_ge(s_idx, 16)
        gather_reps = reps if variant in ('full', 'gather_only') else 1
        dve_reps = reps if variant in ('full', 'dve_only') else 1
        CH = NIDX // NCHUNK  # idxs per gather chunk
        for rep in range(gather_reps):
            for g in range(NG):
                j = rep * NG + g
                buf = gts[j % NBUF]
                if variant == 'full' and j >= NBUF:
                    # wait until the h-min stage (the last gt reader) has
                    # consumed group j-NBUF
                    nc.gpsimd.wait_ge(s_v, j - NBUF + 1)
                for ch in range(NCHUNK):
                    # chunk ch covers group idxs [ch*CH, (ch+1)*CH): same
                    # wrapped-idx cols and same gt cols as one big gather
                    nc.gpsimd.dma_gather(
                        buf[:, ch * CH:(ch + 1) * CH].rearrange(
                            "p (j c) -> p j c", c=CP),
                        table[g * GF * E:(g + 1) * GF * E, :],
                        idx_sb[:, g * (NIDX // 16) + ch * (CH // 16):
                               g * (NIDX // 16) + (ch + 1) * (CH // 16)],
                        CH, CH, CP, single_packet=True,
                        queue_num=ch % 4,
                    ).then_inc(s_g[g], 16)

        # --- vector: software-pipelined chain, no per-iteration drains.
        # Gather layout per group: cols = (4h, 2B', 8f, 128c); every op below
        # is a flat contiguous 2D slice (bf16 2x-mode friendly).
        nc.vector.wait_ge(s_msk, 16)

        def dve_iter(rep, k):
            # stage 1 first: h-pair mins from the gathered tile (group k);
            # then_inc on the 2nd op releases the gather buffer ASAP
            if '1' in STAGES and k < NG:
                j = rep * NG + k
                buf = gts[j % NBUF] if variant == 'full' else gts[0]
                nc.vector.wait_ge(s_g[k], 16 * NCHUNK * (rep + 1)
                                  if variant == 'full' else 16 * NCHUNK)
                nc.vector.tensor_tensor(
                    mAs[k % 4][:, :HB], buf[:, :HB], buf[:, 2 * HB:3 * HB], mn)
                nc.vector.tensor_tensor(
                    mAs[k % 4][:, HB:], buf[:, HB:2 * HB],
                    buf[:, 3 * HB:], mn).then_inc(s_v, 1)
            # stage 5: acc += red  (group k-4)
            if '5' in STAGES and 0 <= k - 4 < NG:
                nc.vector.tensor_tensor(
                    acc[:, :], acc[:, :], reds[(k - 4) % 2][:, :], ad)
            # stage 4: add-tree over f  (group k-3): 1024 + 512 + 256 cols,
            # all contiguous inner slices
            if '4' in STAGES and 0 <= k - 3 < NG:
                p = (k - 3) % 2
                rm = rms[p][:, :].rearrange("p (q t x) -> p q t x", q=BQ, t=2)
                nc.vector.tensor_tensor(
                    u1s[p][:, :].rearrange("p (q x) -> p q x", q=BQ),
                    rm[:, :, 0], rm[:, :, 1], ad)
                u1 = u1s[p][:, :].rearrange("p (q t x) -> p q t x", q=BQ, t=2)
                nc.vector.tensor_tensor(
                    u2s[p][:, :].rearrange("p (q x) -> p q x", q=BQ),
                    u1[:, :, 0], u1[:, :, 1], ad)
                u2 = u2s[p][:, :].rearrange("p (q t x) -> p q t x", q=BQ, t=2)
                nc.vector.tensor_tensor(
                    reds[p][:, :].rearrange("p (q x) -> p q x", q=BQ),
                    u2[:, :, 0], u2[:, :, 1], ad)
            # stage 3: respm = (min >= 0) * mask  (group k-2), one 2048-col op
            if '3' in STAGES and 0 <= k - 2 < NG:
                g2 = k - 2
                nc.vector.scalar_tensor_tensor(
                    rms[g2 % 2][:, :], mBs[g2 % 2][:, :], 0.0,
                    mask_sb[:, g2 * QCOLS:(g2 + 1) * QCOLS],
                    mybir.AluOpType.is_ge, mybir.AluOpType.mult)
            # stage 2: min of the two h-pair minima  (group k-1)
            if '2' in STAGES and 0 <= k - 1 < NG:
                g1 = k - 1
                nc.vector.tensor_tensor(
                    mBs[g1 % 2][:, :], mAs[g1 % 4][:, :HB],
                    mAs[g1 % 4][:, HB:], mn)

        for rep in range(dve_reps):
            nc.vector.memset(acc[:, :], 0.0)
            for k in range(NIT):
                dve_iter(rep, k)
        nc.vector.drain().then_inc(s_f, 1)

        # --- sync: write the partial result back -------------------------
        nc.sync.wait_ge(s_f, 1)
        nc.sync.dma_start(out_acc[:, :], acc[:, :]).then_inc(s_f, 16)
        nc.sync.wait_ge(s_f, 17)
    nc.finalize()
    return nc


def _get_nc(reps=1, variant='full'):
    key = (reps, variant)
    if key not in _NC:
        _NC[key] = _build_nc(reps, variant)
    return _NC[key]


def _hashed_indices(x, thresholds, hash_values, input_order):
    """idx[b, f, h] in [0, E) — the H3 hash of the binarized inputs."""
    bits = (x[:, :, None] >= thresholds[None, :, :])
    bits = bits.reshape(B, IB)[:, input_order].astype(np.int32)
    hin = bits.reshape(B, F, U)
    prod = hin[:, :, None, :] * hash_values[None, None, :, :].astype(np.int32)
    return np.bitwise_xor.reduce(prod, axis=-1)  # [B, F, H]


def _shard_inputs(idx, data, mask):
    """Per-core input dicts: table slab, wrapped gather indices, repl. mask."""
    data_t = np.zeros((F, E, CP), dtype=ml_dtypes.bfloat16)
    data_t[:, :, :C] = np.transpose(data, (1, 2, 0)).astype(ml_dtypes.bfloat16)
    in_maps = []
    for k in range(NCORES):
        fs = k * FPC
        table_k = np.ascontiguousarray(data_t[fs:fs + FPC]).reshape(FPC * E, CP)
        # gather order within a group: i = ((h*BQ + B')*GF + f_local)*128 + p
        # (h outermost so every DVE op is a flat contiguous slice)
        idxk = idx[:, fs:fs + FPC, :]                        # [B, FPC, H]
        r = np.empty((NG, NIDX), np.int32)
        offs = np.arange(GF, dtype=np.int32) * E
        for g in range(NG):
            sub = idxk[:, g * GF:(g + 1) * GF, :]            # [B, GF, H]
            a = sub.reshape(BQ, 128, GF, H).transpose(3, 0, 2, 1)  # [H,BQ,GF,128]
            r[g] = (a + offs[None, None, :, None]).reshape(NIDX)
        iw16 = np.zeros((16, NG * (NIDX // 16)), np.int16)
        for g in range(NG):
            iw16[:, g * (NIDX // 16):(g + 1) * (NIDX // 16)] = (
                r[g].reshape(NIDX // 16, 16).T.astype(np.int16))
        iw = np.tile(iw16, (8, 1))  # replicated per Q7 core group
        # mask tiled to the (g, B', f, c) col layout of the stt stage, bf16
        mk = np.zeros((FPC, CP), np.float32)
        mk[:, :C] = mask[:, fs:fs + FPC].T
        m1 = mk.reshape(NG, 1, GF * CP)
        m2 = np.broadcast_to(m1, (NG, BQ, GF * CP)).reshape(1, NG * BQ * GF * CP)
        mrep = np.ascontiguousarray(np.broadcast_to(
            m2, (128, NG * BQ * GF * CP))).astype(ml_dtypes.bfloat16)
        in_maps.append({"table": table_k, "idxw": iw, "maskr": mrep})
    return in_maps


def kernel(x, thresholds, data, hash_values, input_order, mask, bias):
    import os
    from concourse.bass_utils import run_bass_kernel_spmd

    x = np.asarray(x, np.float32)
    thresholds = np.asarray(thresholds, np.float32)
    data = np.asarray(data, np.float32)
    hash_values = np.asarray(hash_values, np.int32)
    input_order = np.asarray(input_order, np.int32)
    mask = np.asarray(mask, np.float32)
    bias = np.asarray(bias, np.float32)

    idx = _hashed_indices(x, thresholds, hash_values, input_order)
    in_maps = _shard_inputs(idx, data, mask)

    trace = bool(int(os.environ.get("WISARD_TRACE", "0")))
    res = run_bass_kernel_spmd(_get_nc(), in_maps, core_ids=list(range(NCORES)),
                               trace=trace)
    if trace and res.exec_time_ns is not None:
        kernel.last_exec_time_ns = res.exec_time_ns
        kernel.last_trace = res.instructions_and_trace
    kernel.last_results = res

    out = np.zeros((B, CP), np.float32)
    for r in res.results:
        out += r["out_acc"].reshape(128, BQ, CP).transpose(1, 0, 2).reshape(B, CP)
    return out[:, :C] + bias[None, :].astype(np.float32)


# revision 21
# speedup vs baseline: 1.4071x; 1.1185x over previous
"""BackpropWiSARD forward on 8 Trainium2 NeuronCores.

  out[b,c] = sum_f mask[c,f] * min_h [data[c, f, idx[b,f,h]] >= 0] + bias[c]

Fast path (mask == ones, which is what setup_inputs produces):
- Only the SIGN of data matters. Host packs sign bytes s[f,e,c] = (data<0)
  into 128B rows (one per (f,e), 100 classes + pad), laid at 256B stride in
  HBM. The filter axis F=512 is sharded 64-per-core; each core DMA-gathers
  the B*H*64 rows it needs (elem_size=128B, stride=256B — the 256B-multiple
  elem restriction is transpose-only in the ucode, so the instruction is
  emitted directly), 8.4MB/core instead of 16.8MB.
- On chip everything stays byte-packed: OR over the H=4 hash lookups
  (bitwise_or in uint32 = 4 classes per element), then a SWAR add-tree over
  filters (byte counts <= 64 never carry across lanes), accumulating
  n_neg[b,c] = #filters with any negative lookup. One uint32 [128,64] tile
  is written back; host computes out = F - sum_cores(n_neg).
- Indices load per-group via SP-engine HWDGE so gathers start immediately.

General-mask fallback: the bf16 gather kernel (min over h, binarize*mask,
add-tree over f, f32 accumulate).
"""

import numpy as np
import ml_dtypes

B = 256      # batch
NI = 1024    # num inputs
C = 100      # classes
U = 16       # unit inputs
E = 2048     # unit entries
H = 4        # hashes
BPI = 8      # bits per input
IB = NI * BPI          # 8192
F = IB // U            # 512 filters
NCORES = 8
FPC = F // NCORES      # 64 filters per core
CP = 128               # padded class dim
GF = 8                 # filters per gather group (HW dma_gather limit: 8192 idxs)
NG = FPC // GF         # 8 groups
NIDX = GF * H * B      # 8192 gathered rows per group
BQ = B // 128          # 2 partition-blocks of the batch
import os as _os
ROWB = int(_os.environ.get("WISARD_ROWB", "128"))  # gathered bytes per row
RSTRIDE = 256          # HBM row stride (stride_bytes_256 field is x256)
SP = bool(int(_os.environ.get("WISARD_SP", "1")))  # single_packet on gathers
NCH = int(_os.environ.get("WISARD_NCHUNK", "2"))   # gather chunks per group
STD = bool(int(_os.environ.get("WISARD_STD", "0")))  # standard dma_gather call
if STD:
    ROWB = 256

_NC = {}


def _dma_gather_strided(eng, mybir, out_ap, in_ap, idxs_ap, num_idxs,
                        elem_size, stride_bytes, single_packet, queue_num):
    """dma_gather with elem_size < 256B (the %256 restriction is
    transpose-mode-only in the ucode); emits InstDMAGatherAnt directly."""
    eng._assert_queue_num(queue_num)
    assert idxs_ap.dtype == mybir.dt.int16
    assert stride_bytes % 256 == 0 and stride_bytes // 256 < 256
    _in = eng.lower_ap_dma(in_ap, for_custom_bir_dma=True)
    _idx = eng.lower_ap(idxs_ap)
    _out = eng.lower_ap(out_ap)
    return eng.add_instruction(
        mybir.InstDMAGatherAnt(
            name=eng.bass.get_next_instruction_name(),
            ins=[*_in, _idx, eng.lower_val_access(eng.to_reg(num_idxs))],
            outs=[_out],
            transpose=False,
            num_idxs=num_idxs,
            elem_size=elem_size,
            stride_bytes_256=stride_bytes // 256,
            gen_mode=0,
            single_packet=single_packet,
            queue_num=queue_num,
            sbuf_tokens_per_rank=0,
            sbuf_free_dim_per_rank=0,
            sbuf_free_dim_pad_per_rank=0,
            sbuf_byte_offset=0,
        ))


def _build_nc_fast(reps=1, variant='full'):
    from contextlib import ExitStack
    import concourse.bacc as bacc
    import concourse.mybir as mybir

    nc = bacc.Bacc("TRN2", target_bir_lowering=False, debug=False,
                   num_devices=NCORES, dynamic_dma_scratch_size=32768,
                   num_swdge_queues=4)
    if STD:
        table = nc.dram_tensor("table", [FPC * E, CP], mybir.dt.bfloat16,
                               kind="ExternalInput")
    else:
        table = nc.dram_tensor("table", [FPC * E, RSTRIDE], mybir.dt.uint8,
                               kind="ExternalInput")
    idxw = nc.dram_tensor("idxw", [128, NG * (NIDX // 16)], mybir.dt.int16,
                          kind="ExternalInput")
    out_acc = nc.dram_tensor("out_acc", [128, BQ * CP // 2], mybir.dt.uint16,
                             kind="ExternalOutput")
    DBG = bool(int(_os.environ.get("WISARD_DEBUG", "0")))
    if DBG:
        dbg_gt = nc.dram_tensor("dbg_gt", [128, NIDX // 128 * ROWB],
                                mybir.dt.uint8, kind="ExternalOutput")
        dbg_orf = nc.dram_tensor("dbg_orf", [128, 512], mybir.dt.uint32,
                                 kind="ExternalOutput")
        dbg_red = nc.dram_tensor("dbg_red", [128, 128], mybir.dt.uint16,
                                 kind="ExternalOutput")

    b_or = mybir.AluOpType.bitwise_or
    ad = mybir.AluOpType.add
    u32 = mybir.dt.uint32
    u16 = mybir.dt.uint16
    RW = ROWB // 4             # uint32 words per gathered row (32 or 64)
    UW = 32                    # useful words per row (128 sign bytes)
    NR = NIDX // 128           # 64 rows per partition per group
    HB = (NR // 4) * UW        # 512 useful u32 cols per h-block
    with ExitStack() as sem_stack:
        ent = sem_stack.enter_context
        idx_sb = ent(nc.sbuf_tensor("idx_sb", [128, NG * (NIDX // 16)], mybir.dt.int16))
        gts = [ent(nc.sbuf_tensor(f"gt{i}", [128, NIDX // 128 * ROWB], mybir.dt.uint8))
               for i in range(8)]
        t0 = ent(nc.sbuf_tensor("t0", [128, HB], u32))
        t1 = ent(nc.sbuf_tensor("t1", [128, HB], u32))
        orf = ent(nc.sbuf_tensor("orf", [128, HB], u32))
        a1 = ent(nc.sbuf_tensor("a1", [128, HB], u16))
        a2 = ent(nc.sbuf_tensor("a2", [128, HB // 2], u16))
        red = ent(nc.sbuf_tensor("red", [128, HB // 4], u16))
        acc = ent(nc.sbuf_tensor("acc", [128, BQ * CP // 2], u16))
        s_v = ent(nc.semaphore("s_v"))
        s_f = ent(nc.semaphore("s_f"))
        s_g = [sem_stack.enter_context(nc.semaphore(f"s_g{g}")) for g in range(NG)]
        s_ig = [sem_stack.enter_context(nc.semaphore(f"s_ig{g}")) for g in range(NG)]
        NBUF = len(gts)

        # --- input loads: per-group idx slices via SP-engine HWDGE -------
        # (own semaphore per slice: completion order across loads is not
        # guaranteed, a shared counter would let gathers race the load)
        ISL = NIDX // 16
        for g in range(NG):
            nc.sync.dma_start(idx_sb[:, g * ISL:(g + 1) * ISL],
                              idxw[:, g * ISL:(g + 1) * ISL]).then_inc(s_ig[g], 16)

        # --- gpsimd: one gather instruction per group --------------------
        # (Bacc auto-inserts the GPSIMD 'mlp' library load for dma_gather)
        gather_reps = reps if variant in ('full', 'gather_only') else 1
        dve_reps = reps if variant in ('full', 'dve_only') else 1
        NOGATHER = bool(int(_os.environ.get("WISARD_NOGATHER", "0")))
        CH = NIDX // NCH
        for rep in range(gather_reps if not NOGATHER else 0):
            for g in range(NG):
                j = rep * NG + g
                if variant == 'full' and j >= NBUF:
                    nc.gpsimd.wait_ge(s_v, j - NBUF + 1)
                nc.gpsimd.wait_ge(s_ig[g], 16)
                for ch in range(NCH):
                    dst = gts[j % NBUF][:, ch * CH // 128 * ROWB:
                                        (ch + 1) * CH // 128 * ROWB]
                    idxs = idx_sb[:, g * ISL + ch * (CH // 16):
                                  g * ISL + (ch + 1) * (CH // 16)]
                    qn = (g * NCH + ch) % 4
                    if STD:
                        nc.gpsimd.dma_gather(
                            dst.bitcast(mybir.dt.bfloat16).rearrange(
                                "p (j c) -> p j c", c=CP),
                            table[g * GF * E:(g + 1) * GF * E, :],
                            idxs, CH, CH, CP, single_packet=SP,
                            queue_num=qn,
                        ).then_inc(s_g[g], 16)
                    else:
                        _dma_gather_strided(
                            nc.gpsimd, mybir,
                            dst.rearrange("p (j c) -> p j c", c=ROWB),
                            table[g * GF * E:(g + 1) * GF * E, :ROWB],
                            idxs, CH, ROWB, RSTRIDE, SP, qn,
                        ).then_inc(s_g[g], 16)

        # --- vector: per group, OR over h then SWAR byte-add over f ------
        # group buffer as u32 [128, 2048]: cols = (4h, 16 rows of (q,f), 32w)
        for rep in range(dve_reps):
            nc.vector.memset(acc[:, :], 0)
            for k in range(NG):
                j = rep * NG + k
                buf = gts[j % NBUF] if variant == 'full' else gts[0]
                if not NOGATHER:
                    nc.vector.wait_ge(s_g[k], 16 * NCH * (rep + 1)
                                      if variant == 'full' else 16 * NCH)
                b32 = buf[:, :].bitcast(u32)
                if RW == UW:
                    hblk = [b32[:, m * HB:(m + 1) * HB] for m in range(4)]
                    t0o, t1o = t0[:, :], t1[:, :]
                else:
                    bv = b32.rearrange("p (s w) -> p s w", w=RW)
                    hblk = [bv[:, m * (NR // 4):(m + 1) * (NR // 4), :UW]
                            for m in range(4)]
                    t0o = t0[:, :].rearrange("p (s w) -> p s w", w=UW)
                    t1o = t1[:, :].rearrange("p (s w) -> p s w", w=UW)
                nc.vector.tensor_tensor(t0o, hblk[0], hblk[2], b_or)
                nc.vector.tensor_tensor(
                    t1o, hblk[1], hblk[3], b_or).then_inc(s_v, 1)
                nc.vector.tensor_tensor(orf[:, :], t0[:, :], t1[:, :], b_or)
                v = orf[:, :].bitcast(u16).rearrange(
                    "p (q t x) -> p q t x", q=BQ, t=2)
                nc.vector.tensor_tensor(
                    a1[:, :].rearrange("p (q x) -> p q x", q=BQ),
                    v[:, :, 0], v[:, :, 1], ad)
                v1 = a1[:, :].rearrange("p (q t x) -> p q t x", q=BQ, t=2)
                nc.vector.tensor_tensor(
                    a2[:, :].rearrange("p (q x) -> p q x", q=BQ),
                    v1[:, :, 0], v1[:, :, 1], ad)
                v2 = a2[:, :].rearrange("p (q t x) -> p q t x", q=BQ, t=2)
                nc.vector.tensor_tensor(
                    red[:, :].rearrange("p (q x) -> p q x", q=BQ),
                    v2[:, :, 0], v2[:, :, 1], ad)
                nc.vector.tensor_tensor(acc[:, :], acc[:, :], red[:, :], ad)
        nc.vector.drain().then_inc(s_f, 1)

        # --- sync: write the n_neg counts back ---------------------------
        nc.sync.wait_ge(s_f, 1)
        nc.sync.dma_start(out_acc[:, :], acc[:, :]).then_inc(s_f, 16)
        if DBG:
            nc.sync.dma_start(dbg_gt[:, :], gts[0][:, :]).then_inc(s_f, 16)
            nc.sync.dma_start(dbg_orf[:, :], orf[:, :]).then_inc(s_f, 16)
            nc.sync.dma_start(dbg_red[:, :], red[:, :]).then_inc(s_f, 16)
            nc.sync.wait_ge(s_f, 65)
        else:
            nc.sync.wait_ge(s_f, 17)
    nc.finalize()
    return nc


def _build_nc_masked(reps=1, variant='full'):
    """General-mask fallback: bf16 rows, min over h, binarize*mask, f-tree."""
    from contextlib import ExitStack
    import concourse.bacc as bacc
    import concourse.mybir as mybir

    nc = bacc.Bacc("TRN2", target_bir_lowering=False, debug=False,
                   num_devices=NCORES, dynamic_dma_scratch_size=32768,
                   num_swdge_queues=4)
    table = nc.dram_tensor("table", [FPC * E, CP], mybir.dt.bfloat16,
                           kind="ExternalInput")
    idxw = nc.dram_tensor("idxw", [128, NG * (NIDX // 16)], mybir.dt.int16,
                          kind="ExternalInput")
    maskr = nc.dram_tensor("maskr", [128, NG * BQ * GF * CP], mybir.dt.bfloat16,
                           kind="ExternalInput")
    out_acc = nc.dram_tensor("out_acc", [128, BQ * CP], mybir.dt.float32,
                             kind="ExternalOutput")

    mn = mybir.AluOpType.min
    ad = mybir.AluOpType.add
    NCHUNK = 8
    with ExitStack() as sem_stack:
        ent = sem_stack.enter_context
        idx_sb = ent(nc.sbuf_tensor("idx_sb", [128, NG * (NIDX // 16)], mybir.dt.int16))
        mask_sb = ent(nc.sbuf_tensor("mask_sb", [128, NG * BQ * GF * CP], mybir.dt.bfloat16))
        gts = [ent(nc.sbuf_tensor(f"gt{i}", [128, NIDX], mybir.dt.bfloat16))
               for i in range(4)]
        mAs = [ent(nc.sbuf_tensor(f"mA{i}", [128, 2 * BQ * GF * CP], mybir.dt.bfloat16))
               for i in range(4)]
        mBs = [ent(nc.sbuf_tensor(f"mB{i}", [128, BQ * GF * CP], mybir.dt.bfloat16))
               for i in range(2)]
        rms = [ent(nc.sbuf_tensor(f"rm{i}", [128, BQ * GF * CP], mybir.dt.bfloat16))
               for i in range(2)]
        u1s = [ent(nc.sbuf_tensor(f"u1{i}", [128, BQ * (GF // 2) * CP], mybir.dt.bfloat16))
               for i in range(2)]
        u2s = [ent(nc.sbuf_tensor(f"u2{i}", [128, BQ * (GF // 4) * CP], mybir.dt.bfloat16))
               for i in range(2)]
        reds = [ent(nc.sbuf_tensor(f"red{i}", [128, BQ * CP], mybir.dt.float32))
                for i in range(2)]
        acc = ent(nc.sbuf_tensor("acc", [128, BQ * CP], mybir.dt.float32))
        s_idx = ent(nc.semaphore("s_idx"))
        s_msk = ent(nc.semaphore("s_msk"))
        s_v = ent(nc.semaphore("s_v"))
        s_f = ent(nc.semaphore("s_f"))
        s_g = [sem_stack.enter_context(nc.semaphore(f"s_g{g}")) for g in range(NG)]
        NBUF = len(gts)
        GCOLS = GF * CP
        QCOLS = BQ * GCOLS
        HB = QCOLS
        NIT = NG + 4

        nc.sync.dma_start(idx_sb[:, :], idxw[:, :]).then_inc(s_idx, 16)
        nc.sync.dma_start(mask_sb[:, :], maskr[:, :]).then_inc(s_msk, 16)

        nc.gpsimd.wait_ge(s_idx, 16)
        gather_reps = reps if variant in ('full', 'gather_only') else 1
        dve_reps = reps if variant in ('full', 'dve_only') else 1
        CH = NIDX // NCHUNK
        for rep in range(gather_reps):
            for g in range(NG):
                j = rep * NG + g
                buf = gts[j % NBUF]
                if variant == 'full' and j >= NBUF:
                    nc.gpsimd.wait_ge(s_v, j - NBUF + 1)
                for ch in range(NCHUNK):
                    nc.gpsimd.dma_gather(
                        buf[:, ch * CH:(ch + 1) * CH].rearrange(
                            "p (j c) -> p j c", c=CP),
                        table[g * GF * E:(g + 1) * GF * E, :],
                        idx_sb[:, g * (NIDX // 16) + ch * (CH // 16):
                               g * (NIDX // 16) + (ch + 1) * (CH // 16)],
                        CH, CH, CP, single_packet=True,
                        queue_num=ch % 4,
                    ).then_inc(s_g[g], 16)

        nc.vector.wait_ge(s_msk, 16)

        def dve_iter(rep, k):
            if k < NG:
                j = rep * NG + k
                buf = gts[j % NBUF] if variant == 'full' else gts[0]
                nc.vector.wait_ge(s_g[k], 16 * NCHUNK * (rep + 1)
                                  if variant == 'full' else 16 * NCHUNK)
                nc.vector.tensor_tensor(
                    mAs[k % 4][:, :HB], buf[:, :HB], buf[:, 2 * HB:3 * HB], mn)
                nc.vector.tensor_tensor(
                    mAs[k % 4][:, HB:], buf[:, HB:2 * HB],
                    buf[:, 3 * HB:], mn).then_inc(s_v, 1)
            if 0 <= k - 4 < NG:
                nc.vector.tensor_tensor(
                    acc[:, :], acc[:, :], reds[(k - 4) % 2][:, :], ad)
            if 0 <= k - 3 < NG:
                p = (k - 3) % 2
                rm = rms[p][:, :].rearrange("p (q t x) -> p q t x", q=BQ, t=2)
                nc.vector.tensor_tensor(
                    u1s[p][:, :].rearrange("p (q x) -> p q x", q=BQ),
                    rm[:, :, 0], rm[:, :, 1], ad)
                u1 = u1s[p][:, :].rearrange("p (q t x) -> p q t x", q=BQ, t=2)
                nc.vector.tensor_tensor(
                    u2s[p][:, :].rearrange("p (q x) -> p q x", q=BQ),
                    u1[:, :, 0], u1[:, :, 1], ad)
                u2 = u2s[p][:, :].rearrange("p (q t x) -> p q t x", q=BQ, t=2)
                nc.vector.tensor_tensor(
                    reds[p][:, :].rearrange("p (q x) -> p q x", q=BQ),
                    u2[:, :, 0], u2[:, :, 1], ad)
            if 0 <= k - 2 < NG:
                g2 = k - 2
                nc.vector.scalar_tensor_tensor(
                    rms[g2 % 2][:, :], mBs[g2 % 2][:, :], 0.0,
                    mask_sb[:, g2 * QCOLS:(g2 + 1) * QCOLS],
                    mybir.AluOpType.is_ge, mybir.AluOpType.mult)
            if 0 <= k - 1 < NG:
                g1 = k - 1
                nc.vector.tensor_tensor(
                    mBs[g1 % 2][:, :], mAs[g1 % 4][:, :HB],
                    mAs[g1 % 4][:, HB:], mn)

        for rep in range(dve_reps):
            nc.vector.memset(acc[:, :], 0.0)
            for k in range(NIT):
                dve_iter(rep, k)
        nc.vector.drain().then_inc(s_f, 1)

        nc.sync.wait_ge(s_f, 1)
        nc.sync.dma_start(out_acc[:, :], acc[:, :]).then_inc(s_f, 16)
        nc.sync.wait_ge(s_f, 17)
    nc.finalize()
    return nc


def _get_nc(kind='fast', reps=1, variant='full'):
    key = (kind, reps, variant)
    if key not in _NC:
        b = _build_nc_fast if kind == 'fast' else _build_nc_masked
        _NC[key] = b(reps, variant)
    return _NC[key]


def _hashed_indices(x, thresholds, hash_values, input_order):
    """idx[b, f, h] in [0, E) — the H3 hash of the binarized inputs."""
    bits = (x[:, :, None] >= thresholds[None, :, :])
    bits = bits.reshape(B, IB)[:, input_order].astype(np.int32)
    hin = bits.reshape(B, F, U)
    prod = hin[:, :, None, :] * hash_values[None, None, :, :].astype(np.int32)
    return np.bitwise_xor.reduce(prod, axis=-1)  # [B, F, H]


def _wrap_idx(idxk):
    """[B, FPC, H] hash indices -> wrapped int16 gather streams [128, NG*512].

    gather order within a group: i = ((h*BQ + B')*GF + f_local)*128 + p
    (h outermost so every DVE op is a flat contiguous slice)."""
    r = np.empty((NG, NIDX), np.int32)
    offs = np.arange(GF, dtype=np.int32) * E
    for g in range(NG):
        sub = idxk[:, g * GF:(g + 1) * GF, :]                  # [B, GF, H]
        a = sub.reshape(BQ, 128, GF, H).transpose(3, 0, 2, 1)  # [H,BQ,GF,128]
        r[g] = (a + offs[None, None, :, None]).reshape(NIDX)
    iw16 = np.zeros((16, NG * (NIDX // 16)), np.int16)
    for g in range(NG):
        iw16[:, g * (NIDX // 16):(g + 1) * (NIDX // 16)] = (
            r[g].reshape(NIDX // 16, 16).T.astype(np.int16))
    return np.tile(iw16, (8, 1))  # replicated per Q7 core group


def _shard_inputs_fast(idx, data):
    """Per-core inputs: sign-byte table rows (128B at 256B stride), indices."""
    signs = (data < 0).astype(np.uint8)          # [C, F, E]
    tr = np.transpose(signs, (1, 2, 0))          # [F, E, C]
    in_maps = []
    for k in range(NCORES):
        fs = k * FPC
        slab = np.zeros((FPC * E, RSTRIDE), np.uint8)
        slab[:, :C] = tr[fs:fs + FPC].reshape(FPC * E, C)
        if STD:
            slab = slab.view(ml_dtypes.bfloat16)
        iw = _wrap_idx(idx[:, fs:fs + FPC, :])
        in_maps.append({"table": slab, "idxw": iw})
    return in_maps


def _shard_inputs_masked(idx, data, mask):
    data_t = np.zeros((F, E, CP), dtype=ml_dtypes.bfloat16)
    data_t[:, :, :C] = np.transpose(data, (1, 2, 0)).astype(ml_dtypes.bfloat16)
    in_maps = []
    for k in range(NCORES):
        fs = k * FPC
        table_k = np.ascontiguousarray(data_t[fs:fs + FPC]).reshape(FPC * E, CP)
        iw = _wrap_idx(idx[:, fs:fs + FPC, :])
        mk = np.zeros((FPC, CP), np.float32)
        mk[:, :C] = mask[:, fs:fs + FPC].T
        m1 = mk.reshape(NG, 1, GF * CP)
        m2 = np.broadcast_to(m1, (NG, BQ, GF * CP)).reshape(1, NG * BQ * GF * CP)
        mrep = np.ascontiguousarray(np.broadcast_to(
            m2, (128, NG * BQ * GF * CP))).astype(ml_dtypes.bfloat16)
        in_maps.append({"table": table_k, "idxw": iw, "maskr": mrep})
    return in_maps


def kernel(x, thresholds, data, hash_values, input_order, mask, bias):
    import os
    from concourse.bass_utils import run_bass_kernel_spmd

    x = np.asarray(x, np.float32)
    thresholds = np.asarray(thresholds, np.float32)
    data = np.asarray(data, np.float32)
    hash_values = np.asarray(hash_values, np.int32)
    input_order = np.asarray(input_order, np.int32)
    mask = np.asarray(mask, np.float32)
    bias = np.asarray(bias, np.float32)

    idx = _hashed_indices(x, thresholds, hash_values, input_order)
    fast = bool(np.all(mask == 1.0))
    trace = bool(int(os.environ.get("WISARD_TRACE", "0")))

    if fast:
        in_maps = _shard_inputs_fast(idx, data)
        res = run_bass_kernel_spmd(_get_nc('fast'), in_maps,
                                   core_ids=list(range(NCORES)), trace=trace)
    else:
        in_maps = _shard_inputs_masked(idx, data, mask)
        res = run_bass_kernel_spmd(_get_nc('masked'), in_maps,
                                   core_ids=list(range(NCORES)), trace=trace)
    if trace and res.exec_time_ns is not None:
        kernel.last_exec_time_ns = res.exec_time_ns
        kernel.last_trace = res.instructions_and_trace
    kernel.last_results = res

    if fast:
        n_neg = np.zeros((128, BQ, CP), np.int64)
        for r in res.results:
            n_neg += r["out_acc"].view(np.uint8).reshape(128, BQ, CP)
        resp = F - n_neg.transpose(1, 0, 2).reshape(B, CP)  # [B, CP]
        return resp[:, :C].astype(np.float32) + bias[None, :]
    out = np.zeros((B, CP), np.float32)
    for r in res.results:
        out += r["out_acc"].reshape(128, BQ, CP).transpose(1, 0, 2).reshape(B, CP)
    return out[:, :C] + bias[None, :].astype(np.float32)


# revision 24
# speedup vs baseline: 1.4267x; 1.0140x over previous
"""BackpropWiSARD forward on 8 Trainium2 NeuronCores.

  out[b,c] = sum_f mask[c,f] * min_h [data[c, f, idx[b,f,h]] >= 0] + bias[c]

Fast path (mask == ones, which is what setup_inputs produces):
- Only the SIGN of data matters. Host packs sign bytes s[f,e,c] = (data<0)
  into 128B rows (one per (f,e), 100 classes + pad), laid at 256B stride in
  HBM. The filter axis F=512 is sharded 64-per-core; each core DMA-gathers
  the B*H*64 rows it needs (elem_size=128B, stride=256B — the 256B-multiple
  elem restriction is transpose-only in the ucode, so the instruction is
  emitted directly), 8.4MB/core instead of 16.8MB.
- On chip everything stays byte-packed: OR over the H=4 hash lookups
  (bitwise_or in uint32 = 4 classes per element), then a SWAR add-tree over
  filters (byte counts <= 64 never carry across lanes), accumulating
  n_neg[b,c] = #filters with any negative lookup. One uint32 [128,64] tile
  is written back; host computes out = F - sum_cores(n_neg).
- Indices load per-group via SP-engine HWDGE so gathers start immediately.

General-mask fallback: the bf16 gather kernel (min over h, binarize*mask,
add-tree over f, f32 accumulate).
"""

import numpy as np
import ml_dtypes

B = 256      # batch
NI = 1024    # num inputs
C = 100      # classes
U = 16       # unit inputs
E = 2048     # unit entries
H = 4        # hashes
BPI = 8      # bits per input
IB = NI * BPI          # 8192
F = IB // U            # 512 filters
NCORES = 8
FPC = F // NCORES      # 64 filters per core
CP = 128               # padded class dim
GF = 8                 # filters per gather group (HW dma_gather limit: 8192 idxs)
NG = FPC // GF         # 8 groups
NIDX = GF * H * B      # 8192 gathered rows per group
BQ = B // 128          # 2 partition-blocks of the batch
import os as _os
ROWB = int(_os.environ.get("WISARD_ROWB", "128"))  # gathered bytes per row
RSTRIDE = 256          # HBM row stride (stride_bytes_256 field is x256)
SP = bool(int(_os.environ.get("WISARD_SP", "1")))  # single_packet on gathers
NCH = int(_os.environ.get("WISARD_NCHUNK", "2"))   # gather chunks per group
STD = bool(int(_os.environ.get("WISARD_STD", "0")))  # standard dma_gather call
if STD:
    ROWB = 256

_NC = {}


def _dma_gather_strided(eng, mybir, out_ap, in_ap, idxs_ap, num_idxs,
                        elem_size, stride_bytes, single_packet, queue_num):
    """dma_gather with elem_size < 256B (the %256 restriction is
    transpose-mode-only in the ucode); emits InstDMAGatherAnt directly."""
    eng._assert_queue_num(queue_num)
    assert idxs_ap.dtype == mybir.dt.int16
    assert stride_bytes % 256 == 0 and stride_bytes // 256 < 256
    _in = eng.lower_ap_dma(in_ap, for_custom_bir_dma=True)
    _idx = eng.lower_ap(idxs_ap)
    _out = eng.lower_ap(out_ap)
    return eng.add_instruction(
        mybir.InstDMAGatherAnt(
            name=eng.bass.get_next_instruction_name(),
            ins=[*_in, _idx, eng.lower_val_access(eng.to_reg(num_idxs))],
            outs=[_out],
            transpose=False,
            num_idxs=num_idxs,
            elem_size=elem_size,
            stride_bytes_256=stride_bytes // 256,
            gen_mode=0,
            single_packet=single_packet,
            queue_num=queue_num,
            sbuf_tokens_per_rank=0,
            sbuf_free_dim_per_rank=0,
            sbuf_free_dim_pad_per_rank=0,
            sbuf_byte_offset=0,
        ))


def _build_nc_fast(reps=1, variant='full'):
    from contextlib import ExitStack
    import concourse.bacc as bacc
    import concourse.mybir as mybir

    nc = bacc.Bacc("TRN2", target_bir_lowering=False, debug=False,
                   num_devices=NCORES, dynamic_dma_scratch_size=32768,
                   num_swdge_queues=4)
    # table rows are sign BYTES; declared bf16 (the ISA dtype path is only
    # exercised with 16-bit dtypes) — elem_size is in bf16 elements.
    table = nc.dram_tensor("table", [FPC * E, RSTRIDE // 2], mybir.dt.bfloat16,
                           kind="ExternalInput")
    idxw = nc.dram_tensor("idxw", [128, NG * (NIDX // 16)], mybir.dt.int16,
                          kind="ExternalInput")
    out_acc = nc.dram_tensor("out_acc", [128, BQ * CP // 2], mybir.dt.uint16,
                             kind="ExternalOutput")
    DBG = bool(int(_os.environ.get("WISARD_DEBUG", "0")))
    if DBG:
        dbg_gt = nc.dram_tensor("dbg_gt", [128, NIDX // 128 * ROWB],
                                mybir.dt.uint8, kind="ExternalOutput")
        dbg_orf = nc.dram_tensor("dbg_orf", [128, 512], mybir.dt.uint32,
                                 kind="ExternalOutput")
        dbg_red = nc.dram_tensor("dbg_red", [128, 128], mybir.dt.uint16,
                                 kind="ExternalOutput")

    b_or = mybir.AluOpType.bitwise_or
    ad = mybir.AluOpType.add
    u32 = mybir.dt.uint32
    u16 = mybir.dt.uint16
    RW = ROWB // 4             # uint32 words per gathered row (32 or 64)
    UW = 32                    # useful words per row (128 sign bytes)
    NR = NIDX // 128           # 64 rows per partition per group
    HB = (NR // 4) * UW        # 512 useful u32 cols per h-block
    with ExitStack() as sem_stack:
        ent = sem_stack.enter_context
        idx_sb = ent(nc.sbuf_tensor("idx_sb", [128, NG * (NIDX // 16)], mybir.dt.int16))
        gts = [ent(nc.sbuf_tensor(f"gt{i}", [128, NIDX // 128 * ROWB], mybir.dt.uint8))
               for i in range(8)]
        t0 = ent(nc.sbuf_tensor("t0", [128, HB], u32))
        t1 = ent(nc.sbuf_tensor("t1", [128, HB], u32))
        orf = ent(nc.sbuf_tensor("orf", [128, HB], u32))
        a1 = ent(nc.sbuf_tensor("a1", [128, HB], u16))
        a2 = ent(nc.sbuf_tensor("a2", [128, HB // 2], u16))
        red = ent(nc.sbuf_tensor("red", [128, HB // 4], u16))
        acc = ent(nc.sbuf_tensor("acc", [128, BQ * CP // 2], u16))
        s_v = ent(nc.semaphore("s_v"))
        s_f = ent(nc.semaphore("s_f"))
        s_g = [sem_stack.enter_context(nc.semaphore(f"s_g{g}")) for g in range(NG)]
        s_ig = [sem_stack.enter_context(nc.semaphore(f"s_ig{g}")) for g in range(NG)]
        NBUF = len(gts)

        # --- input loads: per-group idx slices via SP-engine HWDGE -------
        # (own semaphore per slice: completion order across loads is not
        # guaranteed, a shared counter would let gathers race the load)
        ISL = NIDX // 16
        for g in range(NG):
            nc.sync.dma_start(idx_sb[:, g * ISL:(g + 1) * ISL],
                              idxw[:, g * ISL:(g + 1) * ISL]).then_inc(s_ig[g], 16)

        # --- gpsimd: one gather instruction per group --------------------
        # (Bacc auto-inserts the GPSIMD 'mlp' library load for dma_gather)
        gather_reps = reps if variant in ('full', 'gather_only') else 1
        dve_reps = reps if variant in ('full', 'dve_only') else 1
        NOGATHER = bool(int(_os.environ.get("WISARD_NOGATHER", "0")))
        CH = NIDX // NCH
        for rep in range(gather_reps if not NOGATHER else 0):
            for g in range(NG):
                j = rep * NG + g
                if variant == 'full' and j >= NBUF:
                    nc.gpsimd.wait_ge(s_v, j - NBUF + 1)
                nc.gpsimd.wait_ge(s_ig[g], 16)
                for ch in range(NCH):
                    dst = gts[j % NBUF][:, ch * CH // 128 * ROWB:
                                        (ch + 1) * CH // 128 * ROWB]
                    idxs = idx_sb[:, g * ISL + ch * (CH // 16):
                                  g * ISL + (ch + 1) * (CH // 16)]
                    qn = (g * NCH + ch) % 4
                    if STD:
                        nc.gpsimd.dma_gather(
                            dst.bitcast(mybir.dt.bfloat16).rearrange(
                                "p (j c) -> p j c", c=CP),
                            table[g * GF * E:(g + 1) * GF * E, :],
                            idxs, CH, CH, CP, single_packet=SP,
                            queue_num=qn,
                        ).then_inc(s_g[g], 16)
                    else:
                        _dma_gather_strided(
                            nc.gpsimd, mybir,
                            dst.bitcast(mybir.dt.bfloat16).rearrange(
                                "p (j c) -> p j c", c=ROWB // 2),
                            table[g * GF * E:(g + 1) * GF * E, :ROWB // 2],
                            idxs, CH, ROWB // 2, RSTRIDE, SP, qn,
                        ).then_inc(s_g[g], 16)

        # --- vector: per group, OR over h then SWAR byte-add over f ------
        # group buffer as u32 [128, 2048]: cols = (4h, 16 rows of (q,f), 32w)
        for rep in range(dve_reps):
            nc.vector.memset(acc[:, :], 0)
            for k in range(NG):
                j = rep * NG + k
                buf = gts[j % NBUF] if variant == 'full' else gts[0]
                if not NOGATHER:
                    nc.vector.wait_ge(s_g[k], 16 * NCH * (rep + 1)
                                      if variant == 'full' else 16 * NCH)
                b32 = buf[:, :].bitcast(u32)
                if RW == UW:
                    hblk = [b32[:, m * HB:(m + 1) * HB] for m in range(4)]
                    t0o, t1o = t0[:, :], t1[:, :]
                else:
                    bv = b32.rearrange("p (s w) -> p s w", w=RW)
                    hblk = [bv[:, m * (NR // 4):(m + 1) * (NR // 4), :UW]
                            for m in range(4)]
                    t0o = t0[:, :].rearrange("p (s w) -> p s w", w=UW)
                    t1o = t1[:, :].rearrange("p (s w) -> p s w", w=UW)
                nc.vector.tensor_tensor(t0o, hblk[0], hblk[2], b_or)
                nc.vector.tensor_tensor(
                    t1o, hblk[1], hblk[3], b_or).then_inc(s_v, 1)
                nc.vector.tensor_tensor(orf[:, :], t0[:, :], t1[:, :], b_or)
                v = orf[:, :].bitcast(u16).rearrange(
                    "p (q t x) -> p q t x", q=BQ, t=2)
                nc.vector.tensor_tensor(
                    a1[:, :].rearrange("p (q x) -> p q x", q=BQ),
                    v[:, :, 0], v[:, :, 1], ad)
                v1 = a1[:, :].rearrange("p (q t x) -> p q t x", q=BQ, t=2)
                nc.vector.tensor_tensor(
                    a2[:, :].rearrange("p (q x) -> p q x", q=BQ),
                    v1[:, :, 0], v1[:, :, 1], ad)
                v2 = a2[:, :].rearrange("p (q t x) -> p q t x", q=BQ, t=2)
                nc.vector.tensor_tensor(
                    red[:, :].rearrange("p (q x) -> p q x", q=BQ),
                    v2[:, :, 0], v2[:, :, 1], ad)
                nc.vector.tensor_tensor(acc[:, :], acc[:, :], red[:, :], ad)
        nc.vector.drain().then_inc(s_f, 1)

        # --- sync: write the n_neg counts back ---------------------------
        nc.sync.wait_ge(s_f, 1)
        nc.sync.dma_start(out_acc[:, :], acc[:, :]).then_inc(s_f, 16)
        if DBG:
            nc.sync.dma_start(dbg_gt[:, :], gts[0][:, :]).then_inc(s_f, 16)
            nc.sync.dma_start(dbg_orf[:, :], orf[:, :]).then_inc(s_f, 16)
            nc.sync.dma_start(dbg_red[:, :], red[:, :]).then_inc(s_f, 16)
            nc.sync.wait_ge(s_f, 65)
        else:
            nc.sync.wait_ge(s_f, 17)
    nc.finalize()
    return nc


def _build_nc_masked(reps=1, variant='full'):
    """General-mask fallback: bf16 rows, min over h, binarize*mask, f-tree."""
    from contextlib import ExitStack
    import concourse.bacc as bacc
    import concourse.mybir as mybir

    nc = bacc.Bacc("TRN2", target_bir_lowering=False, debug=False,
                   num_devices=NCORES, dynamic_dma_scratch_size=32768,
                   num_swdge_queues=4)
    table = nc.dram_tensor("table", [FPC * E, CP], mybir.dt.bfloat16,
                           kind="ExternalInput")
    idxw = nc.dram_tensor("idxw", [128, NG * (NIDX // 16)], mybir.dt.int16,
                          kind="ExternalInput")
    maskr = nc.dram_tensor("maskr", [128, NG * BQ * GF * CP], mybir.dt.bfloat16,
                           kind="ExternalInput")
    out_acc = nc.dram_tensor("out_acc", [128, BQ * CP], mybir.dt.float32,
                             kind="ExternalOutput")

    mn = mybir.AluOpType.min
    ad = mybir.AluOpType.add
    NCHUNK = 8
    with ExitStack() as sem_stack:
        ent = sem_stack.enter_context
        idx_sb = ent(nc.sbuf_tensor("idx_sb", [128, NG * (NIDX // 16)], mybir.dt.int16))
        mask_sb = ent(nc.sbuf_tensor("mask_sb", [128, NG * BQ * GF * CP], mybir.dt.bfloat16))
        gts = [ent(nc.sbuf_tensor(f"gt{i}", [128, NIDX], mybir.dt.bfloat16))
               for i in range(4)]
        mAs = [ent(nc.sbuf_tensor(f"mA{i}", [128, 2 * BQ * GF * CP], mybir.dt.bfloat16))
               for i in range(4)]
        mBs = [ent(nc.sbuf_tensor(f"mB{i}", [128, BQ * GF * CP], mybir.dt.bfloat16))
               for i in range(2)]
        rms = [ent(nc.sbuf_tensor(f"rm{i}", [128, BQ * GF * CP], mybir.dt.bfloat16))
               for i in range(2)]
        u1s = [ent(nc.sbuf_tensor(f"u1{i}", [128, BQ * (GF // 2) * CP], mybir.dt.bfloat16))
               for i in range(2)]
        u2s = [ent(nc.sbuf_tensor(f"u2{i}", [128, BQ * (GF // 4) * CP], mybir.dt.bfloat16))
               for i in range(2)]
        reds = [ent(nc.sbuf_tensor(f"red{i}", [128, BQ * CP], mybir.dt.float32))
                for i in range(2)]
        acc = ent(nc.sbuf_tensor("acc", [128, BQ * CP], mybir.dt.float32))
        s_idx = ent(nc.semaphore("s_idx"))
        s_msk = ent(nc.semaphore("s_msk"))
        s_v = ent(nc.semaphore("s_v"))
        s_f = ent(nc.semaphore("s_f"))
        s_g = [sem_stack.enter_context(nc.semaphore(f"s_g{g}")) for g in range(NG)]
        NBUF = len(gts)
        GCOLS = GF * CP
        QCOLS = BQ * GCOLS
        HB = QCOLS
        NIT = NG + 4

        nc.sync.dma_start(idx_sb[:, :], idxw[:, :]).then_inc(s_idx, 16)
        nc.sync.dma_start(mask_sb[:, :], maskr[:, :]).then_inc(s_msk, 16)

        nc.gpsimd.wait_ge(s_idx, 16)
        gather_reps = reps if variant in ('full', 'gather_only') else 1
        dve_reps = reps if variant in ('full', 'dve_only') else 1
        CH = NIDX // NCHUNK
        for rep in range(gather_reps):
            for g in range(NG):
                j = rep * NG + g
                buf = gts[j % NBUF]
                if variant == 'full' and j >= NBUF:
                    nc.gpsimd.wait_ge(s_v, j - NBUF + 1)
                for ch in range(NCHUNK):
                    nc.gpsimd.dma_gather(
                        buf[:, ch * CH:(ch + 1) * CH].rearrange(
                            "p (j c) -> p j c", c=CP),
                        table[g * GF * E:(g + 1) * GF * E, :],
                        idx_sb[:, g * (NIDX // 16) + ch * (CH // 16):
                               g * (NIDX // 16) + (ch + 1) * (CH // 16)],
                        CH, CH, CP, single_packet=True,
                        queue_num=ch % 4,
                    ).then_inc(s_g[g], 16)

        nc.vector.wait_ge(s_msk, 16)

        def dve_iter(rep, k):
            if k < NG:
                j = rep * NG + k
                buf = gts[j % NBUF] if variant == 'full' else gts[0]
                nc.vector.wait_ge(s_g[k], 16 * NCHUNK * (rep + 1)
                                  if variant == 'full' else 16 * NCHUNK)
                nc.vector.tensor_tensor(
                    mAs[k % 4][:, :HB], buf[:, :HB], buf[:, 2 * HB:3 * HB], mn)
                nc.vector.tensor_tensor(
                    mAs[k % 4][:, HB:], buf[:, HB:2 * HB],
                    buf[:, 3 * HB:], mn).then_inc(s_v, 1)
            if 0 <= k - 4 < NG:
                nc.vector.tensor_tensor(
                    acc[:, :], acc[:, :], reds[(k - 4) % 2][:, :], ad)
            if 0 <= k - 3 < NG:
                p = (k - 3) % 2
                rm = rms[p][:, :].rearrange("p (q t x) -> p q t x", q=BQ, t=2)
                nc.vector.tensor_tensor(
                    u1s[p][:, :].rearrange("p (q x) -> p q x", q=BQ),
                    rm[:, :, 0], rm[:, :, 1], ad)
                u1 = u1s[p][:, :].rearrange("p (q t x) -> p q t x", q=BQ, t=2)
                nc.vector.tensor_tensor(
                    u2s[p][:, :].rearrange("p (q x) -> p q x", q=BQ),
                    u1[:, :, 0], u1[:, :, 1], ad)
                u2 = u2s[p][:, :].rearrange("p (q t x) -> p q t x", q=BQ, t=2)
                nc.vector.tensor_tensor(
                    reds[p][:, :].rearrange("p (q x) -> p q x", q=BQ),
                    u2[:, :, 0], u2[:, :, 1], ad)
            if 0 <= k - 2 < NG:
                g2 = k - 2
                nc.vector.scalar_tensor_tensor(
                    rms[g2 % 2][:, :], mBs[g2 % 2][:, :], 0.0,
                    mask_sb[:, g2 * QCOLS:(g2 + 1) * QCOLS],
                    mybir.AluOpType.is_ge, mybir.AluOpType.mult)
            if 0 <= k - 1 < NG:
                g1 = k - 1
                nc.vector.tensor_tensor(
                    mBs[g1 % 2][:, :], mAs[g1 % 4][:, :HB],
                    mAs[g1 % 4][:, HB:], mn)

        for rep in range(dve_reps):
            nc.vector.memset(acc[:, :], 0.0)
            for k in range(NIT):
                dve_iter(rep, k)
        nc.vector.drain().then_inc(s_f, 1)

        nc.sync.wait_ge(s_f, 1)
        nc.sync.dma_start(out_acc[:, :], acc[:, :]).then_inc(s_f, 16)
        nc.sync.wait_ge(s_f, 17)
    nc.finalize()
    return nc


def _get_nc(kind='fast', reps=1, variant='full'):
    key = (kind, reps, variant)
    if key not in _NC:
        b = _build_nc_fast if kind == 'fast' else _build_nc_masked
        _NC[key] = b(reps, variant)
    return _NC[key]


def _hashed_indices(x, thresholds, hash_values, input_order):
    """idx[b, f, h] in [0, E) — the H3 hash of the binarized inputs."""
    bits = (x[:, :, None] >= thresholds[None, :, :])
    bits = bits.reshape(B, IB)[:, input_order].astype(np.int32)
    hin = bits.reshape(B, F, U)
    prod = hin[:, :, None, :] * hash_values[None, None, :, :].astype(np.int32)
    return np.bitwise_xor.reduce(prod, axis=-1)  # [B, F, H]


def _wrap_idx(idxk):
    """[B, FPC, H] hash indices -> wrapped int16 gather streams [128, NG*512].

    gather order within a group: i = ((h*BQ + B')*GF + f_local)*128 + p
    (h outermost so every DVE op is a flat contiguous slice)."""
    r = np.empty((NG, NIDX), np.int32)
    offs = np.arange(GF, dtype=np.int32) * E
    for g in range(NG):
        sub = idxk[:, g * GF:(g + 1) * GF, :]                  # [B, GF, H]
        a = sub.reshape(BQ, 128, GF, H).transpose(3, 0, 2, 1)  # [H,BQ,GF,128]
        r[g] = (a + offs[None, None, :, None]).reshape(NIDX)
    iw16 = np.zeros((16, NG * (NIDX // 16)), np.int16)
    for g in range(NG):
        iw16[:, g * (NIDX // 16):(g + 1) * (NIDX // 16)] = (
            r[g].reshape(NIDX // 16, 16).T.astype(np.int16))
    return np.tile(iw16, (8, 1))  # replicated per Q7 core group


def _shard_inputs_fast(idx, data):
    """Per-core inputs: sign-byte table rows (128B at 256B stride), indices."""
    signs = (data < 0).astype(np.uint8)          # [C, F, E]
    tr = np.transpose(signs, (1, 2, 0))          # [F, E, C]
    in_maps = []
    for k in range(NCORES):
        fs = k * FPC
        slab = np.zeros((FPC * E, RSTRIDE), np.uint8)
        slab[:, :C] = tr[fs:fs + FPC].reshape(FPC * E, C)
        slab = slab.view(ml_dtypes.bfloat16)
        iw = _wrap_idx(idx[:, fs:fs + FPC, :])
        in_maps.append({"table": slab, "idxw": iw})
    return in_maps


def _shard_inputs_masked(idx, data, mask):
    data_t = np.zeros((F, E, CP), dtype=ml_dtypes.bfloat16)
    data_t[:, :, :C] = np.transpose(data, (1, 2, 0)).astype(ml_dtypes.bfloat16)
    in_maps = []
    for k in range(NCORES):
        fs = k * FPC
        table_k = np.ascontiguousarray(data_t[fs:fs + FPC]).reshape(FPC * E, CP)
        iw = _wrap_idx(idx[:, fs:fs + FPC, :])
        mk = np.zeros((FPC, CP), np.float32)
        mk[:, :C] = mask[:, fs:fs + FPC].T
        m1 = mk.reshape(NG, 1, GF * CP)
        m2 = np.broadcast_to(m1, (NG, BQ, GF * CP)).reshape(1, NG * BQ * GF * CP)
        mrep = np.ascontiguousarray(np.broadcast_to(
            m2, (128, NG * BQ * GF * CP))).astype(ml_dtypes.bfloat16)
        in_maps.append({"table": table_k, "idxw": iw, "maskr": mrep})
    return in_maps


def kernel(x, thresholds, data, hash_values, input_order, mask, bias):
    import os
    from concourse.bass_utils import run_bass_kernel_spmd

    x = np.asarray(x, np.float32)
    thresholds = np.asarray(thresholds, np.float32)
    data = np.asarray(data, np.float32)
    hash_values = np.asarray(hash_values, np.int32)
    input_order = np.asarray(input_order, np.int32)
    mask = np.asarray(mask, np.float32)
    bias = np.asarray(bias, np.float32)

    idx = _hashed_indices(x, thresholds, hash_values, input_order)
    fast = bool(np.all(mask == 1.0))
    trace = bool(int(os.environ.get("WISARD_TRACE", "0")))

    if fast:
        in_maps = _shard_inputs_fast(idx, data)
        res = run_bass_kernel_spmd(_get_nc('fast'), in_maps,
                                   core_ids=list(range(NCORES)), trace=trace)
    else:
        in_maps = _shard_inputs_masked(idx, data, mask)
        res = run_bass_kernel_spmd(_get_nc('masked'), in_maps,
                                   core_ids=list(range(NCORES)), trace=trace)
    if trace and res.exec_time_ns is not None:
        kernel.last_exec_time_ns = res.exec_time_ns
        kernel.last_trace = res.instructions_and_trace
    kernel.last_results = res

    if fast:
        n_neg = np.zeros((128, BQ, CP), np.int64)
        for r in res.results:
            n_neg += r["out_acc"].view(np.uint8).reshape(128, BQ, CP)
        resp = F - n_neg.transpose(1, 0, 2).reshape(B, CP)  # [B, CP]
        return resp[:, :C].astype(np.float32) + bias[None, :]
    out = np.zeros((B, CP), np.float32)
    for r in res.results:
        out += r["out_acc"].reshape(128, BQ, CP).transpose(1, 0, 2).reshape(B, CP)
    return out[:, :C] + bias[None, :].astype(np.float32)
